# revision 45
# baseline (speedup 1.0000x reference)
"""Multi-head causal self-attention on 8 Trainium2 NeuronCores.

Problem: x[4,2048,1024] @ w_qkv[1024,3072] -> 16-head causal attention
         -> @ w_out[1024,1024] + b_out.

Sharding (hardcoded): 8 cores = 4 batches x 2 head-groups of 8 heads.
Core c handles batch b = c//2 and heads hg*8..hg*8+8, hg = c%2.
Each core computes a partial output [2048,1024] (its 8 heads pushed
through its w_out row-slice); host sums the two head-group partials per
batch and adds b_out.

Everything computes in fp16 (10 mantissa bits; fp32 PSUM accumulation),
which runs matmuls at full 1 cycle/row PE rate and lands ~7e-4 relative
error vs the fp32 reference.

Device algorithm per core (all "transposed orientation" so the only
transpose needed -- x^T -- is done for free on the host):
  qT/kT [512, 2048] and v (natural [2048, 512]) via fp16 matmuls.
  Per head pair (2 heads = 128 partitions), per 512-wide query chunk:
    scores^T[j,i] for both heads into one 2-bank PSUM tile via
    row-tiled (K=64) matmul pairs; ONE exp per key-tile on ScalarE
    (p^T fp16); causal masking via a precomputed 0/1 mask multiply on
    the diagonal band plus variable-width (narrowed) tiles;
    out^T[d,i] += col-tiled matmuls (PSUM accum over j),
    denom[i]   += ones-vector matmuls (M=1) into shared denom banks
    (4 col-strip rows per bank, zero-established by a dummy matmul).
  att^T (unnormalized) is copied to SBUF immediately (frees PSUM);
  1/denom via one batched DVE reciprocal per bank, broadcast over
  partitions via a DRAM bounce, then in-place multiply into att^T.
  partial = att^T.T @ w_out_slice -> DMA to DRAM.

Emission is software-pipelined per 512-token stage s: QKV(s),
out-projection(s-1), attention(s), so the Tile scheduler overlaps
PE-heavy projection work with ScalarE-heavy softmax work and hides the
softmax-denominator normalization latency.
"""

import os
import sys

import numpy as np

if "/opt/trn_rl_repo" not in sys.path:
    sys.path.insert(0, "/opt/trn_rl_repo")

B, T, C = 4, 2048, 1024
H, D = 16, 64
NCORES = 8
HPC = 8  # heads per core
PAIRS = 4  # head pairs per core
CCH = 8  # contraction chunks over C (1024/128)
ICH = 4  # i (query) chunks of 512
NJT = 16  # j (key) tiles of 128

_CACHE = {}


def _build_program():
    import concourse.mybir as mybir
    import concourse.tile as tile
    from concourse import bacc

    f32 = mybir.dt.float32
    f32r = mybir.dt.float32r
    bf16 = mybir.dt.bfloat16
    f16 = mybir.dt.float16
    EXP = mybir.ActivationFunctionType.Exp
    LOG = mybir.ActivationFunctionType.Ln

    nc = bacc.Bacc(
        "TRN2", target_bir_lowering=False, debug=False, num_devices=NCORES
    )
    xt = nc.dram_tensor("xt", [C, T], f16, kind="ExternalInput").ap()
    # wq/wk arrive column-blocked from the host: [4, C, 128] flattened, so
    # each per-(pair, contraction-chunk) [128, 128] weight block is one
    # contiguous 32KB DMA and the first QK chain unblocks early.
    wq = nc.dram_tensor("wq", [4 * C, 128], f16, kind="ExternalInput").ap()
    wk = nc.dram_tensor("wk", [4 * C, 128], f16, kind="ExternalInput").ap()
    wv = nc.dram_tensor("wv", [C, 512], f16, kind="ExternalInput").ap()
    wo = nc.dram_tensor("wo", [512, C], f16, kind="ExternalInput").ap()
    msk = nc.dram_tensor("msk", [128, 896], f16, kind="ExternalInput").ap()
    out = nc.dram_tensor("out", [T, C], f16, kind="ExternalOutput").ap()

    with tile.TileContext(nc) as tc:
        with (
            tc.tile_pool(name="wpool", bufs=8) as wpool,
            tc.tile_pool(name="wvpool", bufs=1) as wvpool,
            tc.tile_pool(name="wopool", bufs=1) as wopool,
            tc.tile_pool(name="xpool", bufs=4) as xpool,
            tc.tile_pool(name="qkpool", bufs=8) as qkpool,
            tc.tile_pool(name="vpool", bufs=16) as vpool,
            tc.tile_pool(name="apool", bufs=4) as apool,
            tc.tile_pool(name="ppool", bufs=12) as ppool,
            tc.tile_pool(name="cpool", bufs=1) as cpool,
            tc.tile_pool(name="rpool", bufs=4) as rpool,
            tc.tile_pool(name="qpool", bufs=4) as qpool,
            tc.tile_pool(name="opool", bufs=4) as opool,
            tc.tile_pool(name="dpool", bufs=4, space="DRAM") as dpool,
            tc.tile_pool(name="ps_a", bufs=2, space="PSUM") as ps_a,
            tc.tile_pool(name="ps_s", bufs=2, space="PSUM") as ps_s,
            tc.tile_pool(name="ps_o", bufs=1, space="PSUM") as ps_o,
            tc.tile_pool(name="ps_d", bufs=1, space="PSUM") as ps_d,
        ):
            # ---- constants / weights resident in SBUF ----
            mask_sb = cpool.tile([128, 896], f16, name="mask_sb")
            nc.sync.dma_start(out=mask_sb, in_=msk)
            ones_sb = cpool.tile([128, 1], f16, name="ones_sb")
            nc.vector.memset(ones_sb, 1.0)
            zer_sb = cpool.tile([128, 128], f16, name="zer_sb")
            nc.vector.memset(zer_sb, 0.0)
            # Pre-place the activation table load for set 6
            # (natural_log_exp_and_others, covering both Exp and Ln) so
            # the compiler's fixpoint pass sees every path covered and
            # does not thrash between per-function sets (~1.3us/reload).
            nc.scalar.add_instruction(
                mybir.InstLoadActFuncSet(
                    name=nc.get_next_instruction_name(),
                    ins=[],
                    outs=[],
                    act_func_set_id=6,
                )
            )
            scr_sb = cpool.tile([128, 2], f16, name="scr_sb")
            nc.scalar.activation(scr_sb[:, 0:1], ones_sb, EXP)
            nc.scalar.activation(scr_sb[:, 1:2], ones_sb, LOG)

            # All bulk input DMAs are single instructions (Sync-queue issue
            # costs ~600ns each, so instruction count matters): the
            # [8*128, W] DRAM regions land as [128, 8*W] SBUF tiles via a
            # "(a b) c -> b (a c)" access pattern. DMA order: x^T stage 0,
            # then per-pair q/k blocks in chain-consumption order.
            w_sb = {}

            def load_wn(wname, wap, n):
                t = wpool.tile([128, CCH * 128], f16, name=f"{wname}_{n}", tag="w")
                nc.sync.dma_start(
                    out=t.rearrange("b (a c) -> b a c", a=CCH),
                    in_=wap[n * C : (n + 1) * C, :].rearrange(
                        "(a b) c -> b a c", b=128
                    ),
                )
                w_sb[wname, n] = t

            xt_sb = [None] * ICH

            def load_xt(t4):
                xt_t = xpool.tile([128, CCH * 512], f16, name=f"xt_{t4}", tag="xt")
                nc.sync.dma_start(
                    out=xt_t.rearrange("b (a c) -> b a c", a=CCH),
                    in_=xt[:, t4 * 512 : (t4 + 1) * 512].rearrange(
                        "(a b) c -> b a c", b=128
                    ),
                )
                xt_sb[t4] = xt_t

            load_xt(0)
            for n in range(PAIRS):
                load_wn("wq", wq, n)
                load_wn("wk", wk, n)
            wv_all = wvpool.tile([128, CCH * 512], f16, name="wv_all", tag="wv")
            nc.sync.dma_start(
                out=wv_all.rearrange("b (a c) -> b a c", a=CCH),
                in_=wv.rearrange("(a b) c -> b a c", b=128),
            )
            load_xt(1)
            wo_all = wopool.tile([128, 4 * C], f16, name="wo_all", tag="wo")
            nc.sync.dma_start(
                out=wo_all.rearrange("b (a c) -> b a c", a=4),
                in_=wo.rearrange("(a b) c -> b a c", b=128),
            )
            load_xt(2)
            load_xt(3)

            # ---- persistent activations ----
            qT = [
                qkpool.tile([128, T], f16, name=f"qT_{p}", tag="qk")
                for p in range(PAIRS)
            ]
            kT = [
                qkpool.tile([128, T], f16, name=f"kT_{p}", tag="qk")
                for p in range(PAIRS)
            ]
            v_sb = [
                vpool.tile([128, 512], f16, name=f"v_{j}", tag="v")
                for j in range(NJT)
            ]
            att = [
                apool.tile([128, T], f16, name=f"att_{p}", tag="att")
                for p in range(PAIRS)
            ]

            def phase_a(t4):
                """QKV projections for token chunk t4 (512 tokens).

                Chain order q0,k0,q1,k1,... so pair 0's attention (which
                only needs qT[0]/kT[0]) unblocks the ScalarE exp stream as
                early as possible each stage.
                """
                xts = xt_sb[t4]
                for n in range(PAIRS):
                    for wname, dst in (("wq", qT), ("wk", kT)):
                        ps = ps_a.tile([128, 512], f32, name="ps_qk", tag="psA")
                        for cc in range(CCH):
                            nc.tensor.matmul(
                                ps,
                                lhsT=w_sb[wname, n][
                                    :, cc * 128 : (cc + 1) * 128
                                ],
                                rhs=xts[:, cc * 512 : (cc + 1) * 512],
                                start=(cc == 0),
                                stop=(cc == CCH - 1),
                            )
                        nc.vector.tensor_copy(
                            dst[n][:, t4 * 512 : (t4 + 1) * 512], ps
                        )
                for tt in range(4):
                    ps = ps_a.tile([128, 512], f32, name="ps_v", tag="psA")
                    for cc in range(CCH):
                        nc.tensor.matmul(
                            ps,
                            lhsT=xts[
                                :, cc * 512 + tt * 128 : cc * 512 + (tt + 1) * 128
                            ],
                            rhs=wv_all[:, cc * 512 : (cc + 1) * 512],
                            start=(cc == 0),
                            stop=(cc == CCH - 1),
                        )
                    nc.scalar.copy(v_sb[t4 * 4 + tt], ps)

            def phase_b(ic, fillers=()):
                """Attention for query chunk ic (512 queries), all pairs.

                fillers: callables (output-projection chains for the prior
                stage) emitted between pairs so the PE queue has matmul
                work interleaved through this ScalarE-heavy phase instead
                of all of it sitting ahead of the attention slots.
                """
                fillers = list(fillers)
                isl = slice(ic * 512, (ic + 1) * 512)
                njt = 4 * ic + 4
                # Two denominator banks per ic: bank A rows {0,32,64,96} =
                # pairs 0,1; bank B = pairs 2,3. One zeroing matmul each
                # establishes the group and write-ordering.
                dbanks = []
                for g in range(2):
                    bank = ps_d.tile([128, 512], f32, name=f"ps_den{g}", tag="psd")
                    nc.tensor.matmul(
                        bank,
                        lhsT=zer_sb,
                        rhs=mask_sb[:, 0:512],
                        start=True,
                        stop=False,
                        skip_group_check=True,
                    )
                    dbanks.append(bank)

                def norm_group(g):
                    """1/denominators for pairs 2g, 2g+1 -> rdb + in-place mul.

                    1/den = exp(-ln(den)) computed entirely on ScalarE
                    (which reads the PSUM bank directly): two ACT ops
                    replace the DVE bank copy + 8-cyc/elem reciprocal +
                    DRAM reshape bounce, and the broadcast DMAs read the
                    result straight out of SBUF.
                    """
                    bank = dbanks[g]
                    rec = rpool.tile([128, 512], f32, name="rec", tag="rec")
                    nc.scalar.activation(rec[0:97, 0:512], bank[0:97, :], LOG)
                    rec2 = rpool.tile([128, 512], f16, name="rec2", tag="rec2")
                    nc.scalar.activation(
                        rec2[0:97, :], rec[0:97, :], EXP, scale=-1.0
                    )
                    dsc = dpool.tile([4, 512], f16, name="dsc", tag="dsc")
                    nc.sync.dma_start(out=dsc, in_=rec2[0:97:32, :])
                    for lp in range(2):
                        pr = 2 * g + lp
                        # fp16 reciprocal broadcast: half the DMA bytes and
                        # an all-fp16 normalization multiply (2x DVE mode).
                        # lp=1 goes out on the Vector queue so the two
                        # pairs' broadcasts run on parallel DMA queues.
                        dq = nc.sync
                        rdb = rpool.tile([128, 512], f16, name="rdb", tag="rdb")
                        dq.dma_start(
                            out=rdb[0:64, :],
                            in_=dsc[2 * lp : 2 * lp + 1, :].broadcast_to(
                                [64, 512]
                            ),
                        )
                        dq.dma_start(
                            out=rdb[64:128, :],
                            in_=dsc[2 * lp + 1 : 2 * lp + 2, :].broadcast_to(
                                [64, 512]
                            ),
                        )
                        asl = att[pr][:, isl]
                        nc.vector.tensor_mul(asl, asl, rdb)

                for pr in range(PAIRS):
                    ps_out = ps_o.tile([128, 512], f32, name="ps_out", tag="pso")
                    dbank = dbanks[pr // 2]
                    dp0 = 64 * (pr % 2)
                    dp1 = 64 * (pr % 2) + 32
                    # Zero the whole ps_out bank in one matmul: establishes
                    # the accumulation group and a WAW dep ordering it before
                    # both col-tiled sub-chains.
                    nc.tensor.matmul(
                        ps_out,
                        lhsT=zer_sb,
                        rhs=mask_sb[:, 0:512],
                        start=True,
                        stop=False,
                        skip_group_check=True,
                    )
                    pacc0 = qpool.tile([128, 512], f16, name="pacc0", tag="pacc")
                    pacc1 = qpool.tile([128, 512], f16, name="pacc1", tag="pacc")
                    for jt in range(njt):
                        jsl = slice(jt * 128, (jt + 1) * 128)
                        dpos = jt - 4 * ic
                        # Causal: query columns below 128*dpos within this
                        # chunk see none of this key tile; narrow fully.
                        ioff = 128 * dpos if dpos > 0 else 0
                        w = 512 - ioff
                        islw = slice(ic * 512 + ioff, (ic + 1) * 512)
                        sb = ps_s.tile([128, 1024], f32, name="sb", tag="pss")
                        # head1's live region is shifted left by ioff so the
                        # two heads' scores are ADJACENT on diagonal tiles:
                        # one exp covers [ioff, 1024-ioff) with no dead zone.
                        nc.tensor.matmul(
                            sb[:, ioff:512],
                            lhsT=kT[pr][0:64, jsl],
                            rhs=qT[pr][0:64, islw],
                            start=True,
                            stop=True,
                            tile_position=(0, 0),
                        )
                        nc.tensor.matmul(
                            sb[:, 512 : 1024 - ioff],
                            lhsT=kT[pr][64:128, jsl],
                            rhs=qT[pr][64:128, islw],
                            start=True,
                            stop=True,
                            tile_position=(64, 0),
                        )
                        pTb = ppool.tile([128, 1024], f16, name="pTb", tag="pT")
                        nc.scalar.activation(
                            pTb[:, ioff : 1024 - ioff],
                            sb[:, ioff : 1024 - ioff],
                            EXP,
                            scale=0.125,
                        )
                        pT0 = pTb[:, 0:512]
                        # head1 live: pTb cols [512, 512+w) <-> queries
                        # [ioff, 512)
                        pT1 = pTb[:, 512 : 512 + w]
                        if dpos >= 0:
                            msl = mask_sb[:, 384 : 384 + w]
                            nc.vector.tensor_mul(
                                pT0[:, ioff:512], pT0[:, ioff:512], msl
                            )
                            nc.vector.tensor_mul(pT1, pT1, msl)
                        last = jt == njt - 1
                        vt = v_sb[jt]
                        nc.tensor.matmul(
                            ps_out[0:64, ioff:512],
                            lhsT=vt[:, pr * 128 : pr * 128 + 64],
                            rhs=pT0[:, ioff:512],
                            start=False,
                            stop=False,
                            tile_position=(0, 0),
                            skip_group_check=True,
                        )
                        nc.tensor.matmul(
                            ps_out[64:128, ioff:512],
                            lhsT=vt[:, pr * 128 + 64 : pr * 128 + 128],
                            rhs=pT1,
                            start=False,
                            stop=last,
                            tile_position=(0, 64),
                            skip_group_check=True,
                        )
                        if jt == 0:
                            nc.vector.tensor_copy(pacc0, pT0)
                            nc.vector.tensor_copy(pacc1, pT1)
                        else:
                            nc.vector.tensor_add(
                                pacc0[:, ioff:512],
                                pacc0[:, ioff:512],
                                pT0[:, ioff:512],
                            )
                            nc.vector.tensor_add(
                                pacc1[:, ioff:512], pacc1[:, ioff:512], pT1
                            )
                    # Partition-reduce the accumulated p-sums into the
                    # shared denominator bank (2 matmuls instead of 2/key-tile).
                    nc.tensor.matmul(
                        dbank[dp0 : dp0 + 1, :],
                        lhsT=ones_sb,
                        rhs=pacc0,
                        start=False,
                        stop=False,
                        tile_position=(0, dp0),
                        skip_group_check=True,
                    )
                    nc.tensor.matmul(
                        dbank[dp1 : dp1 + 1, :],
                        lhsT=ones_sb,
                        rhs=pacc1,
                        start=False,
                        stop=False,
                        tile_position=(0, dp1),
                        skip_group_check=True,
                    )
                    # Unnormalized copy frees ps_out quickly; normalization
                    # happens in-place on att once the broadcast lands.
                    asl = att[pr][:, isl]
                    nc.vector.tensor_copy(asl, ps_out)
                    if pr % 2 == 1:
                        norm_group(pr // 2)
                    # sprinkle output-projection chains for the previous
                    # stage between pairs
                    nf = len(fillers) // 4
                    for f in fillers[pr * nf : (pr + 1) * nf]:
                        f()
                for f in fillers[4 * (len(fillers) // 4) :]:
                    f()

            def phase_c_chain(tt, n):
                """One output-projection chain: out[tt*128:, n*512:]."""
                tsl = slice(tt * 128, (tt + 1) * 128)

                def emit():
                    ps = ps_a.tile([128, 512], f32, name="ps_c", tag="psA")
                    for fc in range(4):
                        nc.tensor.matmul(
                            ps,
                            lhsT=att[fc][:, tsl],
                            rhs=wo_all[
                                :, fc * C + n * 512 : fc * C + (n + 1) * 512
                            ],
                            start=(fc == 0),
                            stop=(fc == 3),
                        )
                    ost = opool.tile([128, 512], f16, name="ost", tag="ost")
                    nc.vector.tensor_copy(ost, ps)
                    nc.sync.dma_start(
                        out=out[tsl, n * 512 : (n + 1) * 512], in_=ost
                    )

                return emit

            def phase_c_chains(s):
                return [
                    phase_c_chain(tt, n)
                    for tt in range(4 * s, 4 * s + 4)
                    for n in range(2)
                ]

            for s in range(4):
                phase_a(s)
                phase_b(s, phase_c_chains(s - 1) if s >= 1 else ())

            # Final-stage projection: all 8 chains open at once across the
            # freed attention PSUM banks, emitted fc-major so the
            # pair-group-0 matmuls overlap the last norm chain's latency.
            fin_locs = [(tt, n) for tt in range(12, 16) for n in range(2)]
            s2 = ps_s.tile([128, 1024], f32, name="pc_s0", tag="pss")
            s3 = ps_s.tile([128, 1024], f32, name="pc_s1", tag="pss")
            fin_ps = [
                ps_a.tile([128, 512], f32, name="pc0", tag="psA"),
                ps_a.tile([128, 512], f32, name="pc1", tag="psA"),
                s2[:, 0:512],
                s2[:, 512:1024],
                s3[:, 0:512],
                s3[:, 512:1024],
                ps_o.tile([128, 512], f32, name="pc_o", tag="pso"),
                ps_d.tile([128, 512], f32, name="pc_d", tag="psd"),
            ]
            for fc in range(4):
                for i in range(8):
                    tt, n = fin_locs[i]
                    nc.tensor.matmul(
                        fin_ps[i],
                        lhsT=att[fc][:, tt * 128 : (tt + 1) * 128],
                        rhs=wo_all[
                            :, fc * C + n * 512 : fc * C + (n + 1) * 512
                        ],
                        start=(fc == 0),
                        stop=(fc == 3),
                    )
            for i in range(8):
                tt, n = fin_locs[i]
                ost = opool.tile([128, 512], f16, name="ost_f", tag="ost")
                if i % 2 == 0:
                    nc.vector.tensor_copy(ost, fin_ps[i])
                else:
                    nc.scalar.copy(ost, fin_ps[i])
                nc.sync.dma_start(
                    out=out[tt * 128 : (tt + 1) * 128, n * 512 : (n + 1) * 512],
                    in_=ost,
                )

    nc.compile()
    return nc


def _get_program():
    if "nc" not in _CACHE:
        _CACHE["nc"] = _build_program()
    return _CACHE["nc"]


def _make_mask():
    # msk[jj, z] = 1 if z >= jj + 384 else 0; diagonal-position-p mask
    # tile is msk[:, 384-128p : 384-128p+512].
    jj = np.arange(128)[:, None]
    z = np.arange(896)[None, :]
    return (z >= jj + 384).astype(np.float16)


def _make_in_maps(x, w_qkv, w_out):
    mask = _make_mask()
    in_maps = []
    def colblock(w):
        # [C, 512] -> [4, C, 128] -> [4*C, 128]: each pair's column block
        # contiguous for chunked DMA.
        return np.ascontiguousarray(
            w.reshape(C, 4, 128).transpose(1, 0, 2).reshape(4 * C, 128)
        )

    for core in range(NCORES):
        b, hg = core // 2, core % 2
        cs = slice(hg * 512, (hg + 1) * 512)
        f16 = np.float16
        in_maps.append(
            {
                "xt": np.ascontiguousarray(x[b].T).astype(f16),
                "wq": colblock(
                    np.asarray(w_qkv[:, hg * 512 : hg * 512 + 512]).astype(f16)
                ),
                "wk": colblock(
                    np.asarray(
                        w_qkv[:, 1024 + hg * 512 : 1024 + hg * 512 + 512]
                    ).astype(f16)
                ),
                "wv": np.ascontiguousarray(
                    w_qkv[:, 2048 + hg * 512 : 2048 + hg * 512 + 512]
                ).astype(f16),
                "wo": np.ascontiguousarray(w_out[cs, :]).astype(f16),
                "msk": mask,
            }
        )
    return in_maps


def _run_device(in_maps, trace=False):
    from concourse.bass_utils import run_bass_kernel_spmd

    nc = _get_program()
    return run_bass_kernel_spmd(
        nc, in_maps, core_ids=list(range(NCORES)), trace=trace
    )


def kernel(x, w_qkv, w_out, b_out):
    x = np.asarray(x, dtype=np.float32)
    w_qkv = np.asarray(w_qkv, dtype=np.float32)
    w_out = np.asarray(w_out, dtype=np.float32)
    b_out = np.asarray(b_out, dtype=np.float32)

    res = _run_device(_make_in_maps(x, w_qkv, w_out)).results
    out = np.empty((B, T, C), dtype=np.float32)
    for b in range(B):
        out[b] = (
            res[2 * b]["out"].astype(np.float32)
            + res[2 * b + 1]["out"].astype(np.float32)
            + b_out
        )
    return out

